# revision 8
# baseline (speedup 1.0000x reference)
"""Trainium2 Bass kernel for nn_BidirectionalRNN (3-layer LN-tanh RNN, bidir).

Sharding: 8 cores = 4 batch-shards x 2 directions (l2r on cores 0-3, r2l on
cores 4-7), B_loc=32 batches per core. All cores run the same SPMD program;
per-core inputs differ (direction weights + batch slice).

On-core layout: H on partitions as 4 chunks of 128, batch along free dim.
Wavefront emission: at wavefront step tau the three units (t=tau,l=0),
(t=tau-1,l=1), (t=tau-2,l=2) are independent and share batched tau-level ops:

  ps_mega[128, 3, KC, B] (psum) <- per-unit Wh/Wx matmuls (bf16, FWL-friendly)
  st[128, u, k, :B] = ps_mega[u] + (xp(t) | bias12) (3 DVE adds)
  st[.., B:] = Square(st[.., :B])  (one batched ACT op)
  stats rows (psum, partition 0): mean-> sm[0:1, u*B:(u+1)B],
    meansq-> sm[0:1, 96+u*B:...]; each is 4 accumulating matmuls with
    stationary ones/H so the k-fold happens in PSUM.
  batched fast-rsqrt Newton chain on [1, 96] rows -> rc16 [1, 192] f16
    (rstd rows | mean*rstd rows)
  A/C broadcast: 2 matmuls ones16.T @ rc16-row -> AC psum [128, 192]
  y = st*A - C (2 batched DVE ops via stride-0 views); hn = Tanh(y) (one ACT)
  FC for l==2: L[45, t*B:(t+1)*B] = Wfc_half.T @ h2 (+ b_fc on l2r cores)

All matmul operands are 16-bit (no fp32 transposes) so LDWEIGHTS runs with
fast-weight-load. Host combines: logits[b,t] = L_l2r[b,t] + L_r2l[b, idx[b,t]].
"""

import numpy as np
import ml_dtypes

import concourse.bass as bass
import concourse.bacc as bacc
import concourse.tile as tile
from concourse import mybir
from concourse.bass_utils import run_bass_kernel_spmd

BF16 = ml_dtypes.bfloat16
import os
USE_FP16 = os.environ.get("KERNEL_DT16", "f16") == "f16"
NP16 = np.float16 if USE_FP16 else BF16

H = 512
IN_DIM = 300
NCLS = 45
LN_EPS = 1e-5
P = 128
KC = H // P  # 4 chunks
N_CORES = 8

f32 = mybir.dt.float32
f16 = mybir.dt.float16
i32 = mybir.dt.int32
bf16 = mybir.dt.float16 if USE_FP16 else mybir.dt.bfloat16


def _stride0_view(ap, reps, width):
    """[P, width] AP -> [P, reps, width] AP re-reading the same cols."""
    return bass.AP(tensor=ap.tensor, offset=ap.offset,
                   ap=[ap.ap[0], [0, reps], [1, width]])


def _stride0_view_mid(ap, width):
    """[P, KC] AP -> [P, KC, width] AP, broadcasting each col along width."""
    return bass.AP(tensor=ap.tensor, offset=ap.offset,
                   ap=[ap.ap[0], ap.ap[1], [0, width]])


def build_nc(T=256, B=32):
    """Build the SPMD program. B = batches per core."""
    COLS = T * B
    S = min(1024, COLS)          # pre-phase slab width (cols)
    MMN = min(512, S)            # matmul moving width
    n_slabs = COLS // S
    B3 = 3 * B                   # batched stats width

    nc = bacc.Bacc(None, target_bir_lowering=False)

    # ---- DRAM parameters (per-core values supplied via in_maps) ----
    xt_d = nc.dram_tensor("xt", [3, P, COLS], bf16, kind="ExternalInput")
    wemb_d = nc.dram_tensor("wemb", [P, 3, H], bf16, kind="ExternalInput")
    wx0_d = nc.dram_tensor("wx0", [P, KC, H], bf16, kind="ExternalInput")
    # recurrence weights: Wh0, Wx1, Wh1, Wx2, Wh2
    wrec_d = nc.dram_tensor("wrec", [5, P, KC, H], bf16, kind="ExternalInput")
    bias0_d = nc.dram_tensor("bias0", [P, KC], f32, kind="ExternalInput")
    bias12_d = nc.dram_tensor("bias12", [P, 2, KC], f32, kind="ExternalInput")
    wfc_d = nc.dram_tensor("wfc", [P, KC, NCLS], bf16, kind="ExternalInput")
    fcb_d = nc.dram_tensor("fcb", [NCLS, 1], f32, kind="ExternalInput")
    out_d = nc.dram_tensor("out", [NCLS, COLS], f32, kind="ExternalOutput")

    Sq = mybir.ActivationFunctionType.Square
    Tanh = mybir.ActivationFunctionType.Tanh
    Alu = mybir.AluOpType

    with tile.TileContext(nc) as tc:
        import contextlib
        with contextlib.ExitStack() as ctx:
            const = ctx.enter_context(tc.tile_pool(name="const", bufs=1))
            big = ctx.enter_context(tc.tile_pool(name="big", bufs=1))
            xtp = ctx.enter_context(tc.tile_pool(name="xtp", bufs=2))
            xep = ctx.enter_context(tc.tile_pool(name="xep", bufs=2))
            stp = ctx.enter_context(tc.tile_pool(name="stp", bufs=2))
            hp = ctx.enter_context(tc.tile_pool(name="hp", bufs=2))
            yp = ctx.enter_context(tc.tile_pool(name="yp", bufs=2))
            acp = ctx.enter_context(tc.tile_pool(name="acp", bufs=2))
            tiny = ctx.enter_context(tc.tile_pool(name="tiny", bufs=3))
            ps_pre = ctx.enter_context(tc.tile_pool(name="ps_pre", bufs=2, space="PSUM"))
            ps_mega = ctx.enter_context(tc.tile_pool(name="ps_mega", bufs=3, space="PSUM"))
            ps_sm = ctx.enter_context(tc.tile_pool(name="ps_sm", bufs=2, space="PSUM"))

            # ---- constants / weights into SBUF ----
            wemb_sb = const.tile([P, 3, H], bf16)
            nc.sync.dma_start(out=wemb_sb, in_=wemb_d.ap())
            wx0_sb = const.tile([P, KC, H], bf16)
            nc.sync.dma_start(out=wx0_sb, in_=wx0_d.ap())
            wrec_sb = const.tile([P, 5, KC, H], bf16)
            nc.sync.dma_start(out=wrec_sb, in_=wrec_d.ap().rearrange("n p k m -> p n k m"))
            bias0_sb = const.tile([P, KC], f32)
            nc.sync.dma_start(out=bias0_sb, in_=bias0_d.ap())
            wfc_sb = const.tile([P, KC, NCLS], bf16)
            nc.sync.dma_start(out=wfc_sb, in_=wfc_d.ap())
            fcb_sb = const.tile([NCLS, 1], f32)
            nc.sync.dma_start(out=fcb_sb, in_=fcb_d.ap())
            bias12_sb = const.tile([P, 2, KC], f32)
            nc.sync.dma_start(out=bias12_sb, in_=bias12_d.ap())

            ones16 = const.tile([1, P], f16)
            nc.vector.memset(ones16, 1.0)
            sc_ones = const.tile([P, 1], f16)
            nc.vector.memset(sc_ones, 1.0 / H)
            qk32 = const.tile([1, B3], i32)
            nc.vector.memset(qk32, 0x5F3759DF)

            xp_sb = big.tile([P, T, KC, B], bf16)     # xproj0 (+bias0), all steps
            L_sb = big.tile([NCLS, COLS], f32)        # FC accumulator

            # ---- pre-phase: embedding + xproj0, slab by slab ----
            for sl in range(n_slabs):
                c0 = sl * S
                xt_tiles = []
                for k in range(3):
                    xt_t = xtp.tile([P, S], bf16, tag=f"xt{k}")
                    nc.sync.dma_start(out=xt_t, in_=xt_d.ap()[k, :, c0:c0 + S])
                    xt_tiles.append(xt_t)
                xe_tiles = []
                for m in range(KC):
                    xe_t = xep.tile([P, S], bf16, tag=f"xe{m}")
                    xe_tiles.append(xe_t)
                for m in range(KC):
                    for ns in range(S // MMN):
                        pse = ps_pre.tile([P, MMN], f32, tag="pre")
                        for k in range(3):
                            nc.tensor.matmul(pse, wemb_sb[:, k, bass.ts(m, P)],
                                             xt_tiles[k][:, bass.ts(ns, MMN)],
                                             start=(k == 0), stop=(k == 2))
                        nc.scalar.copy(xe_tiles[m][:, bass.ts(ns, MMN)], pse)
                for m in range(KC):
                    for ns in range(S // MMN):
                        psx = ps_pre.tile([P, MMN], f32, tag="pre")
                        for k in range(KC):
                            nc.tensor.matmul(psx, wx0_sb[:, k, bass.ts(m, P)],
                                             xe_tiles[k][:, bass.ts(ns, MMN)],
                                             start=(k == 0), stop=(k == KC - 1))
                        n0 = c0 + ns * MMN
                        t0 = n0 // B
                        nt = MMN // B
                        nc.vector.tensor_scalar(
                            xp_sb[:, t0:t0 + nt, m, :], psx,
                            bias0_sb[:, m:m + 1], None, Alu.add)

            # ---- recurrence (wavefront emission, batched tau-level ops) ----
            # flat layouts: st [P, 768] = raw(3*KC*B=384) | squared(384);
            # hn/y [P, 384]; (l, k) unit block at col l*128 + k*32
            U = KC * B  # 128, one unit's cols
            hn = hp.tile([P, 3 * U], bf16, tag="hn")
            nc.vector.memset(hn, 0.0)

            wh_idx = [0, 2, 4]   # Wh0, Wh1, Wh2 in wrec
            wx_idx = [None, 1, 3]

            def _v2(tile_ap, off):
                """[P, KC, B] strided view at flat col offset."""
                a = tile_ap[:, off:off + U]
                return bass.AP(tensor=a.tensor, offset=a.offset,
                               ap=[a.ap[0], [B, KC], [1, B]])

            for tau in range(T + 2):
                h_prev = hn
                hn = hp.tile([P, 3 * U], bf16, tag="hn")
                ps = ps_mega.tile([P, 3, KC, B], f32, tag="mega")
                st = stp.tile([P, 2 * 3 * U], bf16, tag="st")
                sm = ps_sm.tile([P, 512], f32, tag="sm")

                # per-unit weight matmuls + st add + mean stats
                for l in range(3):
                    t = tau - l
                    tc_ = min(max(t, 0), T - 1)  # clamped (fringe slots are garbage)
                    n_mm = KC * (2 if l > 0 else 1)
                    for m in range(KC):
                        i = 0
                        for k in range(KC):
                            nc.tensor.matmul(ps[:, l, m, :],
                                             wrec_sb[:, wh_idx[l], k, bass.ts(m, P)],
                                             h_prev[:, l * U + k * B:l * U + (k + 1) * B],
                                             start=(i == 0), stop=(i == n_mm - 1))
                            i += 1
                        if l > 0:
                            for k in range(KC):
                                nc.tensor.matmul(ps[:, l, m, :],
                                                 wrec_sb[:, wx_idx[l], k, bass.ts(m, P)],
                                                 h_prev[:, (l - 1) * U + k * B:(l - 1) * U + (k + 1) * B],
                                                 start=False, stop=(i == n_mm - 1))
                                i += 1
                    if l == 0:
                        nc.vector.tensor_tensor(_v2(st, 0), ps[:, 0, :, :],
                                                xp_sb[:, tc_, :, :], Alu.add)
                    else:
                        nc.vector.tensor_tensor(
                            _v2(st, l * U), ps[:, l, :, :],
                            _stride0_view_mid(bias12_sb[:, l - 1, :], B), Alu.add)
                    # mean row: sm[0:1, l*B:(l+1)*B], k-fold via psum accumulation
                    for k in range(KC):
                        nc.tensor.matmul(sm[0:1, l * B:(l + 1) * B],
                                         sc_ones, st[:, l * U + k * B:l * U + (k + 1) * B],
                                         start=(k == 0), stop=(k == KC - 1))

                # batched square + meansq stats rows
                nc.scalar.activation(st[:, 3 * U:6 * U], st[:, 0:3 * U], Sq)
                for l in range(3):
                    for k in range(KC):
                        nc.tensor.matmul(sm[0:1, B3 + l * B:B3 + (l + 1) * B],
                                         sc_ones,
                                         st[:, (3 + l) * U + k * B:(3 + l) * U + (k + 1) * B],
                                         start=(k == 0), stop=(k == KC - 1))

                # batched fast-rsqrt on [1, 96] rows (copy psum stats to SBUF first)
                stM = tiny.tile([1, 2 * B3], f32, tag="stM")
                nc.vector.tensor_scalar(stM, sm[0:1, 0:2 * B3], 0.0, None, Alu.add)
                mean = stM[:, 0:B3]
                msq = stM[:, B3:2 * B3]
                m2 = tiny.tile([1, B3], f32, tag="m2")
                nc.vector.tensor_tensor(m2, mean, mean, Alu.mult)
                ve = tiny.tile([1, B3], f32, tag="ve")
                nc.vector.tensor_tensor(ve, msq, m2, Alu.subtract)
                nc.vector.tensor_scalar(ve, ve, LN_EPS, None, Alu.add)
                ui = tiny.tile([1, B3], i32, tag="ui")
                nc.vector.tensor_scalar(ui, ve.bitcast(i32), 1, None,
                                        Alu.arith_shift_right)
                y0i = tiny.tile([1, B3], i32, tag="y0i")
                nc.vector.tensor_tensor(y0i, qk32, ui, Alu.subtract)
                cur = y0i.bitcast(f32)
                y2 = tiny.tile([1, B3], f32, tag="y2")
                nc.vector.tensor_tensor(y2, cur, cur, Alu.mult)
                xy2 = tiny.tile([1, B3], f32, tag="xy2")
                nc.vector.tensor_tensor(xy2, ve, y2, Alu.mult)
                e = tiny.tile([1, B3], f32, tag="e")
                nc.vector.tensor_scalar(e, xy2, -0.5, 1.5, Alu.mult, Alu.add)
                rc16 = tiny.tile([1, 2 * B3], f16, tag="rc16")
                nc.vector.tensor_tensor(rc16[:, 0:B3], cur, e, Alu.mult)
                cm = tiny.tile([1, B3], f32, tag="cm")
                nc.vector.tensor_tensor(cm, mean, cur, Alu.mult)
                nc.vector.tensor_tensor(rc16[:, B3:2 * B3], cm, e, Alu.mult)

                # broadcast A (rstd) and C (mean*rstd) across partitions via PE
                nc.tensor.matmul(sm[:, 192:192 + B3], ones16, rc16[:, 0:B3],
                                 start=True, stop=True)
                nc.tensor.matmul(sm[:, 192 + B3:192 + 2 * B3], ones16,
                                 rc16[:, B3:2 * B3], start=True, stop=True)
                ac16 = acp.tile([P, 2 * B3], f16, tag="ac")
                nc.scalar.copy(ac16, sm[:, 192:192 + 2 * B3])

                # y = st*A - C (per-unit stride-0 views), hn = tanh(y) batched
                y = yp.tile([P, 3 * U], f16, tag="y")
                for l in range(3):
                    nc.vector.tensor_tensor(
                        _v2(y, l * U), _v2(st, l * U),
                        _stride0_view(ac16[:, l * B:(l + 1) * B], KC, B),
                        Alu.mult)
                    nc.vector.tensor_tensor(
                        _v2(y, l * U), _v2(y, l * U),
                        _stride0_view(ac16[:, B3 + l * B:B3 + (l + 1) * B], KC, B),
                        Alu.subtract)
                nc.scalar.activation(hn, y, Tanh)

                # zero fringe slots whose (t<0) state is read next tau
                if tau == 0:
                    nc.vector.memset(hn[:, U:2 * U], 0.0)
                    nc.vector.memset(hn[:, 2 * U:3 * U], 0.0)
                elif tau == 1:
                    nc.vector.memset(hn[:, 2 * U:3 * U], 0.0)

                # FC for the l=2 unit when valid
                t2 = tau - 2
                if 0 <= t2 < T:
                    for k in range(KC):
                        nc.tensor.matmul(sm[0:45, 384:384 + B], wfc_sb[:, k, :],
                                         hn[:, 2 * U + k * B:2 * U + (k + 1) * B],
                                         start=(k == 0), stop=(k == KC - 1))
                    nc.vector.tensor_scalar(L_sb[:, t2 * B:(t2 + 1) * B],
                                            sm[0:45, 384:384 + B],
                                            fcb_sb, None, Alu.add)

            nc.sync.dma_start(out=out_d.ap(), in_=L_sb)

    nc.compile()
    return nc


# ---------------- host-side prep ----------------

def _lay_w(w):
    """[H, M] fp32 -> [P, KC, M] bf16 chunk layout."""
    Hh, M = w.shape
    kc = Hh // P
    return np.ascontiguousarray(
        w.reshape(kc, P, M).transpose(1, 0, 2)).astype(NP16)


def make_in_maps(inputs, T=256, B=32):
    """Build the 8 per-core input dicts from the full problem inputs."""
    x = np.asarray(inputs["x"], np.float32)[:, :T]
    rx = np.asarray(inputs["reverse_x"], np.float32)[:, :T]
    W_emb = np.asarray(inputs["W_emb"], np.float32)
    b_emb = np.asarray(inputs["b_emb"], np.float32)
    W_fc = np.asarray(inputs["W_fc"], np.float32)
    b_fc = np.asarray(inputs["b_fc"], np.float32)

    wemb_aug = np.zeros((3 * P, H), np.float32)
    wemb_aug[:IN_DIM] = W_emb
    wemb_aug[IN_DIM] = b_emb
    wemb_lay = _lay_w(wemb_aug)  # [P, 3, H]

    dirs = {}
    for d, (xx, sfx, wfc_half, fcb) in enumerate([
            (x, "l2r", W_fc[:H], b_fc),
            (rx, "r2l", W_fc[H:], np.zeros_like(b_fc))]):
        Wx = np.asarray(inputs[f"Wx_{sfx}"], np.float32)
        bx = np.asarray(inputs[f"bx_{sfx}"], np.float32)
        Wh = np.asarray(inputs[f"Wh_{sfx}"], np.float32)
        bh = np.asarray(inputs[f"bh_{sfx}"], np.float32)
        wrec = np.stack([_lay_w(Wh[0]), _lay_w(Wx[1]), _lay_w(Wh[1]),
                         _lay_w(Wx[2]), _lay_w(Wh[2])])  # [5, P, KC, H]
        bias0 = (bx[0] + bh[0]).reshape(KC, P).T.astype(np.float32)  # [P, KC]
        bias12 = np.stack([(bx[1] + bh[1]).reshape(KC, P).T,
                           (bx[2] + bh[2]).reshape(KC, P).T], 1).astype(np.float32)
        dirs[d] = dict(
            x=xx,
            wx0=_lay_w(Wx[0]),
            wrec=np.ascontiguousarray(wrec),
            bias0=np.ascontiguousarray(bias0),
            bias12=np.ascontiguousarray(bias12),
            wfc=_lay_w(wfc_half),
            fcb=fcb.reshape(NCLS, 1).astype(np.float32),
        )

    n_shard = N_CORES // 2
    in_maps = []
    for core in range(N_CORES):
        d = 0 if core < n_shard else 1
        s = core % n_shard
        dd = dirs[d]
        xc = dd["x"][s * B:(s + 1) * B]  # [B, T, IN]
        xa = np.zeros((3 * P, T * B), np.float32)
        xa[:IN_DIM] = xc.transpose(2, 1, 0).reshape(IN_DIM, T * B)
        xa[IN_DIM] = 1.0
        in_maps.append({
            "xt": np.ascontiguousarray(xa.reshape(3, P, T * B)).astype(NP16),
            "wemb": wemb_lay,
            "wx0": dd["wx0"],
            "wrec": dd["wrec"],
            "bias0": dd["bias0"],
            "bias12": dd["bias12"],
            "wfc": dd["wfc"],
            "fcb": dd["fcb"],
        })
    return in_maps


def combine_outputs(results, pad_start_index, T=256, B=32):
    """results: list of 8 dicts with 'out' [NCLS, T*B]. Returns [128*T, NCLS]."""
    n_shard = N_CORES // 2
    Bfull = n_shard * B
    L = np.zeros((2, Bfull, T, NCLS), np.float32)
    for core in range(N_CORES):
        d = 0 if core < n_shard else 1
        s = core % n_shard
        o = results[core]["out"].reshape(NCLS, T, B)  # col = t*B + b
        L[d, s * B:(s + 1) * B] = o.transpose(2, 1, 0)
    p = np.asarray(pad_start_index).astype(np.int64)[:, None]
    j = np.arange(T)[None, :]
    idx = np.where(j < p, p - j - 1, j)  # [Bfull, T]
    L2g = np.take_along_axis(L[1], idx[:, :, None], axis=1)
    logits = L[0] + L2g
    return logits.reshape(Bfull * T, NCLS)


_NC_CACHE = {}


def kernel(**inputs) -> np.ndarray:
    T = int(inputs["max_length"])
    assert T == 256, f"kernel compiled for T=256, got {T}"
    B = 32
    ln_g = np.asarray(inputs["ln_g"], np.float32)
    ln_b = np.asarray(inputs["ln_b"], np.float32)
    assert np.all(ln_g == 1.0) and np.all(ln_b == 0.0), \
        "general ln_g/ln_b path not wired"

    key = (T, B)
    if key not in _NC_CACHE:
        _NC_CACHE[key] = build_nc(T=T, B=B)
    nc = _NC_CACHE[key]

    in_maps = make_in_maps(inputs, T=T, B=B)
    res = run_bass_kernel_spmd(nc, in_maps, list(range(N_CORES)))
    return combine_outputs(res.results, inputs["pad_start_index"], T=T, B=B)


if __name__ == "__main__":
    import reference
    inp = reference.setup_inputs()
    out = kernel(**{k: np.asarray(v) for k, v in inp.items()})
    ref = np.asarray(reference.reference(**inp))
    err = np.abs(out - ref).max() / np.abs(ref).max()
    print(f"Relative error: {err:.3e}")


# revision 11
# speedup vs baseline: 1.0731x; 1.0731x over previous
"""Trainium2 Bass kernel for nn_BidirectionalRNN (3-layer LN-tanh RNN, bidir).

Sharding: 8 cores = 4 batch-shards x 2 directions (l2r on cores 0-3, r2l on
cores 4-7), B_loc=32 batches per core. All cores run the same SPMD program;
per-core inputs differ (direction weights + batch slice).

On-core layout: H on partitions as 4 chunks of 128, batch along free dim.
Wavefront emission: at wavefront step tau the three units (t=tau,l=0),
(t=tau-1,l=1), (t=tau-2,l=2) are independent and share batched tau-level ops:

  ps_mega[128, 3, KC, B] (psum) <- per-unit Wh/Wx matmuls (bf16, FWL-friendly)
  st[128, u, k, :B] = ps_mega[u] + (xp(t) | bias12) (3 DVE adds)
  st[.., B:] = Square(st[.., :B])  (one batched ACT op)
  stats rows (psum, partition 0): mean-> sm[0:1, u*B:(u+1)B],
    meansq-> sm[0:1, 96+u*B:...]; each is 4 accumulating matmuls with
    stationary ones/H so the k-fold happens in PSUM.
  batched fast-rsqrt Newton chain on [1, 96] rows -> rc16 [1, 192] f16
    (rstd rows | mean*rstd rows)
  A/C broadcast: 2 matmuls ones16.T @ rc16-row -> AC psum [128, 192]
  y = st*A - C (2 batched DVE ops via stride-0 views); hn = Tanh(y) (one ACT)
  FC for l==2: L[45, t*B:(t+1)*B] = Wfc_half.T @ h2 (+ b_fc on l2r cores)

All matmul operands are 16-bit (no fp32 transposes) so LDWEIGHTS runs with
fast-weight-load. Host combines: logits[b,t] = L_l2r[b,t] + L_r2l[b, idx[b,t]].
"""

import numpy as np
import ml_dtypes

import concourse.bass as bass
import concourse.bacc as bacc
import concourse.tile as tile
from concourse import mybir
from concourse.bass_utils import run_bass_kernel_spmd

BF16 = ml_dtypes.bfloat16
import os
USE_FP16 = os.environ.get("KERNEL_DT16", "f16") == "f16"
NP16 = np.float16 if USE_FP16 else BF16

H = 512
IN_DIM = 300
NCLS = 45
LN_EPS = 1e-5
P = 128
KC = H // P  # 4 chunks
N_CORES = 8

f32 = mybir.dt.float32
f16 = mybir.dt.float16
i32 = mybir.dt.int32
bf16 = mybir.dt.float16 if USE_FP16 else mybir.dt.bfloat16


def _stride0_view(ap, reps, width):
    """[P, width] AP -> [P, reps, width] AP re-reading the same cols."""
    return bass.AP(tensor=ap.tensor, offset=ap.offset,
                   ap=[ap.ap[0], [0, reps], [1, width]])


def _stride0_view_mid(ap, width):
    """[P, KC] AP -> [P, KC, width] AP, broadcasting each col along width."""
    return bass.AP(tensor=ap.tensor, offset=ap.offset,
                   ap=[ap.ap[0], ap.ap[1], [0, width]])


def build_nc(T=256, B=32):
    """Build the SPMD program. B = batches per core."""
    COLS = T * B
    S = min(1024, COLS)          # pre-phase slab width (cols)
    MMN = min(512, S)            # matmul moving width
    n_slabs = COLS // S
    B3 = 3 * B                   # batched stats width

    nc = bacc.Bacc(None, target_bir_lowering=False)

    # ---- DRAM parameters (per-core values supplied via in_maps) ----
    xt_d = nc.dram_tensor("xt", [3, P, COLS], bf16, kind="ExternalInput")
    wemb_d = nc.dram_tensor("wemb", [P, 3, H], bf16, kind="ExternalInput")
    wx0_d = nc.dram_tensor("wx0", [P, KC, H], bf16, kind="ExternalInput")
    # recurrence weights: Wh0, Wx1, Wh1, Wx2, Wh2
    wrec_d = nc.dram_tensor("wrec", [5, P, KC, H], bf16, kind="ExternalInput")
    bias0_d = nc.dram_tensor("bias0", [P, KC], f32, kind="ExternalInput")
    bias12_d = nc.dram_tensor("bias12", [P, 2, KC], f32, kind="ExternalInput")
    wfc_d = nc.dram_tensor("wfc", [P, KC, NCLS], bf16, kind="ExternalInput")
    fcb_d = nc.dram_tensor("fcb", [NCLS, 1], f32, kind="ExternalInput")
    out_d = nc.dram_tensor("out", [NCLS, COLS], f32, kind="ExternalOutput")

    Sq = mybir.ActivationFunctionType.Square
    Tanh = mybir.ActivationFunctionType.Tanh
    Alu = mybir.AluOpType

    with tile.TileContext(nc) as tc:
        import contextlib
        with contextlib.ExitStack() as ctx:
            const = ctx.enter_context(tc.tile_pool(name="const", bufs=1))
            big = ctx.enter_context(tc.tile_pool(name="big", bufs=1))
            xtp = ctx.enter_context(tc.tile_pool(name="xtp", bufs=2))
            xep = ctx.enter_context(tc.tile_pool(name="xep", bufs=2))
            stp = ctx.enter_context(tc.tile_pool(name="stp", bufs=2))
            hp = ctx.enter_context(tc.tile_pool(name="hp", bufs=2))
            yp = ctx.enter_context(tc.tile_pool(name="yp", bufs=2))
            acp = ctx.enter_context(tc.tile_pool(name="acp", bufs=2))
            tiny = ctx.enter_context(tc.tile_pool(name="tiny", bufs=3))
            ps_pre = ctx.enter_context(tc.tile_pool(name="ps_pre", bufs=2, space="PSUM"))
            ps_mega = ctx.enter_context(tc.tile_pool(name="ps_mega", bufs=2, space="PSUM"))
            ps_sm = ctx.enter_context(tc.tile_pool(name="ps_sm", bufs=1, space="PSUM"))

            # ---- constants / weights into SBUF ----
            wemb_sb = const.tile([P, 3, H], bf16)
            nc.sync.dma_start(out=wemb_sb, in_=wemb_d.ap())
            wx0_sb = const.tile([P, KC, H], bf16)
            nc.sync.dma_start(out=wx0_sb, in_=wx0_d.ap())
            wrec_sb = const.tile([P, 5, KC, H], bf16)
            nc.sync.dma_start(out=wrec_sb, in_=wrec_d.ap().rearrange("n p k m -> p n k m"))
            bias0_sb = const.tile([P, KC], f32)
            nc.sync.dma_start(out=bias0_sb, in_=bias0_d.ap())
            wfc_sb = const.tile([P, KC, NCLS], bf16)
            nc.sync.dma_start(out=wfc_sb, in_=wfc_d.ap())
            fcb_sb = const.tile([NCLS, 1], f32)
            nc.sync.dma_start(out=fcb_sb, in_=fcb_d.ap())
            bias12_sb = const.tile([P, 2, KC], f32)
            nc.sync.dma_start(out=bias12_sb, in_=bias12_d.ap())

            ones16 = const.tile([1, P], f16)
            nc.vector.memset(ones16, 1.0)
            sc_ones = const.tile([P, 1], f16)
            nc.vector.memset(sc_ones, 1.0 / H)
            qk32 = const.tile([1, B3], i32)
            nc.vector.memset(qk32, 0x5F3759DF)

            xp_sb = big.tile([P, T, KC, B], bf16)     # xproj0 (+bias0), all steps
            L_sb = big.tile([NCLS, COLS], f32)        # FC accumulator

            # ---- pre-phase: embedding + xproj0, slab by slab ----
            for sl in range(n_slabs):
                c0 = sl * S
                xt_tiles = []
                for k in range(3):
                    xt_t = xtp.tile([P, S], bf16, tag=f"xt{k}")
                    nc.sync.dma_start(out=xt_t, in_=xt_d.ap()[k, :, c0:c0 + S])
                    xt_tiles.append(xt_t)
                xe_tiles = []
                for m in range(KC):
                    xe_t = xep.tile([P, S], bf16, tag=f"xe{m}")
                    xe_tiles.append(xe_t)
                for m in range(KC):
                    for ns in range(S // MMN):
                        pse = ps_pre.tile([P, MMN], f32, tag="pre")
                        for k in range(3):
                            nc.tensor.matmul(pse, wemb_sb[:, k, bass.ts(m, P)],
                                             xt_tiles[k][:, bass.ts(ns, MMN)],
                                             start=(k == 0), stop=(k == 2))
                        nc.scalar.copy(xe_tiles[m][:, bass.ts(ns, MMN)], pse)
                for m in range(KC):
                    for ns in range(S // MMN):
                        psx = ps_pre.tile([P, MMN], f32, tag="pre")
                        for k in range(KC):
                            nc.tensor.matmul(psx, wx0_sb[:, k, bass.ts(m, P)],
                                             xe_tiles[k][:, bass.ts(ns, MMN)],
                                             start=(k == 0), stop=(k == KC - 1))
                        n0 = c0 + ns * MMN
                        t0 = n0 // B
                        nt = MMN // B
                        nc.vector.tensor_scalar(
                            xp_sb[:, t0:t0 + nt, m, :], psx,
                            bias0_sb[:, m:m + 1], None, Alu.add)

            # ---- recurrence (wavefront emission, batched tau-level ops) ----
            # flat layouts: st [P, 768] = raw(3*KC*B=384) | squared(384);
            # hn/y [P, 384]; (l, k) unit block at col l*128 + k*32
            U = KC * B  # 128, one unit's cols
            hn = hp.tile([P, 3 * U], bf16, tag="hn")
            nc.vector.memset(hn, 0.0)

            wh_idx = [0, 2, 4]   # Wh0, Wh1, Wh2 in wrec
            wx_idx = [None, 1, 3]

            def _v2(tile_ap, off):
                """[P, KC, B] strided view at flat col offset."""
                a = tile_ap[:, off:off + U]
                return bass.AP(tensor=a.tensor, offset=a.offset,
                               ap=[a.ap[0], [B, KC], [1, B]])

            for tau in range(T + 2):
                h_prev = hn
                hn = hp.tile([P, 3 * U], bf16, tag="hn")
                ps = ps_mega.tile([P, 3, KC, B], f32, tag="mega")
                st = stp.tile([P, 2 * 3 * U], bf16, tag="st")
                sm = ps_sm.tile([P, 512], f32, tag="sm")
                psA = ps_sm.tile([P, 3 * U], f32, tag="psA")
                psC = ps_sm.tile([P, 3 * U], f32, tag="psC")

                # all weight matmuls first (keeps the PE queue unblocked)
                for l in range(3):
                    n_mm = KC * (2 if l > 0 else 1)
                    for m in range(KC):
                        i = 0
                        for k in range(KC):
                            nc.tensor.matmul(ps[:, l, m, :],
                                             wrec_sb[:, wh_idx[l], k, bass.ts(m, P)],
                                             h_prev[:, l * U + k * B:l * U + (k + 1) * B],
                                             start=(i == 0), stop=(i == n_mm - 1))
                            i += 1
                        if l > 0:
                            for k in range(KC):
                                nc.tensor.matmul(ps[:, l, m, :],
                                                 wrec_sb[:, wx_idx[l], k, bass.ts(m, P)],
                                                 h_prev[:, (l - 1) * U + k * B:(l - 1) * U + (k + 1) * B],
                                                 start=False, stop=(i == n_mm - 1))
                                i += 1

                # per-unit st add + mean stats
                for l in range(3):
                    t = tau - l
                    tc_ = min(max(t, 0), T - 1)  # clamped (fringe slots are garbage)
                    if l == 0:
                        nc.vector.tensor_tensor(_v2(st, 0), ps[:, 0, :, :],
                                                xp_sb[:, tc_, :, :], Alu.add)
                    else:
                        nc.vector.tensor_tensor(
                            _v2(st, l * U), ps[:, l, :, :],
                            _stride0_view_mid(bias12_sb[:, l - 1, :], B), Alu.add)
                    # mean row: sm[0:1, l*B:(l+1)*B], k-fold via psum accumulation
                    for k in range(KC):
                        nc.tensor.matmul(sm[0:1, l * B:(l + 1) * B],
                                         sc_ones, st[:, l * U + k * B:l * U + (k + 1) * B],
                                         start=(k == 0), stop=(k == KC - 1))

                # batched square + meansq stats rows
                nc.scalar.activation(st[:, 3 * U:6 * U], st[:, 0:3 * U], Sq)
                for l in range(3):
                    for k in range(KC):
                        nc.tensor.matmul(sm[0:1, B3 + l * B:B3 + (l + 1) * B],
                                         sc_ones,
                                         st[:, (3 + l) * U + k * B:(3 + l) * U + (k + 1) * B],
                                         start=(k == 0), stop=(k == KC - 1))

                # batched fast-rsqrt on [1, 96] rows (copy psum stats to SBUF first)
                stM = tiny.tile([1, 2 * B3], f32, tag="stM")
                nc.vector.tensor_scalar(stM, sm[0:1, 0:2 * B3], 0.0, None, Alu.add)
                mean = stM[:, 0:B3]
                msq = stM[:, B3:2 * B3]
                m2 = tiny.tile([1, B3], f32, tag="m2")
                nc.vector.tensor_tensor(m2, mean, mean, Alu.mult)
                ve = tiny.tile([1, B3], f32, tag="ve")
                nc.vector.tensor_tensor(ve, msq, m2, Alu.subtract)
                nc.vector.tensor_scalar(ve, ve, LN_EPS, None, Alu.add)
                ui = tiny.tile([1, B3], i32, tag="ui")
                nc.vector.tensor_scalar(ui, ve.bitcast(i32), 1, None,
                                        Alu.arith_shift_right)
                y0i = tiny.tile([1, B3], i32, tag="y0i")
                nc.vector.tensor_tensor(y0i, qk32, ui, Alu.subtract)
                cur = y0i.bitcast(f32)
                y2 = tiny.tile([1, B3], f32, tag="y2")
                nc.vector.tensor_tensor(y2, cur, cur, Alu.mult)
                xy2 = tiny.tile([1, B3], f32, tag="xy2")
                nc.vector.tensor_tensor(xy2, ve, y2, Alu.mult)
                e = tiny.tile([1, B3], f32, tag="e")
                nc.vector.tensor_scalar(e, xy2, -0.5, 1.5, Alu.mult, Alu.add)
                rc16 = tiny.tile([1, 2 * B3], f16, tag="rc16")
                nc.vector.tensor_tensor(rc16[:, 0:B3], cur, e, Alu.mult)
                cm = tiny.tile([1, B3], f32, tag="cm")
                nc.vector.tensor_tensor(cm, mean, cur, Alu.mult)
                nc.vector.tensor_tensor(rc16[:, B3:2 * B3], cm, e, Alu.mult)

                # broadcast A (rstd) and C (mean*rstd) across partitions via PE,
                # expanded KC-fold via stride-0 moving so y is 2 flat DVE ops
                for l in range(3):
                    av = rc16[:, l * B:(l + 1) * B]
                    av = bass.AP(tensor=av.tensor, offset=av.offset,
                                 ap=[av.ap[0], [0, KC], [1, B]])
                    cv = rc16[:, B3 + l * B:B3 + (l + 1) * B]
                    cv = bass.AP(tensor=cv.tensor, offset=cv.offset,
                                 ap=[cv.ap[0], [0, KC], [1, B]])
                    nc.tensor.matmul(psA[:, l * U:(l + 1) * U], ones16, av,
                                     start=True, stop=True)
                    nc.tensor.matmul(psC[:, l * U:(l + 1) * U], ones16, cv,
                                     start=True, stop=True)

                # y = st*A - C (flat, batched), hn = tanh(y)
                y = yp.tile([P, 3 * U], f16, tag="y")
                nc.vector.tensor_tensor(y, st[:, 0:3 * U], psA, Alu.mult)
                nc.vector.tensor_tensor(y, y, psC, Alu.subtract)
                nc.scalar.activation(hn, y, Tanh)

                # zero fringe slots whose (t<0) state is read next tau
                if tau == 0:
                    nc.vector.memset(hn[:, U:2 * U], 0.0)
                    nc.vector.memset(hn[:, 2 * U:3 * U], 0.0)
                elif tau == 1:
                    nc.vector.memset(hn[:, 2 * U:3 * U], 0.0)

                # FC for the l=2 unit when valid
                t2 = tau - 2
                if 0 <= t2 < T:
                    for k in range(KC):
                        nc.tensor.matmul(sm[0:45, 384:384 + B], wfc_sb[:, k, :],
                                         hn[:, 2 * U + k * B:2 * U + (k + 1) * B],
                                         start=(k == 0), stop=(k == KC - 1))
                    nc.vector.tensor_scalar(L_sb[:, t2 * B:(t2 + 1) * B],
                                            sm[0:45, 384:384 + B],
                                            fcb_sb, None, Alu.add)

            nc.sync.dma_start(out=out_d.ap(), in_=L_sb)

    nc.compile()
    return nc


# ---------------- host-side prep ----------------

def _lay_w(w):
    """[H, M] fp32 -> [P, KC, M] bf16 chunk layout."""
    Hh, M = w.shape
    kc = Hh // P
    return np.ascontiguousarray(
        w.reshape(kc, P, M).transpose(1, 0, 2)).astype(NP16)


def make_in_maps(inputs, T=256, B=32):
    """Build the 8 per-core input dicts from the full problem inputs."""
    x = np.asarray(inputs["x"], np.float32)[:, :T]
    rx = np.asarray(inputs["reverse_x"], np.float32)[:, :T]
    W_emb = np.asarray(inputs["W_emb"], np.float32)
    b_emb = np.asarray(inputs["b_emb"], np.float32)
    W_fc = np.asarray(inputs["W_fc"], np.float32)
    b_fc = np.asarray(inputs["b_fc"], np.float32)

    wemb_aug = np.zeros((3 * P, H), np.float32)
    wemb_aug[:IN_DIM] = W_emb
    wemb_aug[IN_DIM] = b_emb
    wemb_lay = _lay_w(wemb_aug)  # [P, 3, H]

    dirs = {}
    for d, (xx, sfx, wfc_half, fcb) in enumerate([
            (x, "l2r", W_fc[:H], b_fc),
            (rx, "r2l", W_fc[H:], np.zeros_like(b_fc))]):
        Wx = np.asarray(inputs[f"Wx_{sfx}"], np.float32)
        bx = np.asarray(inputs[f"bx_{sfx}"], np.float32)
        Wh = np.asarray(inputs[f"Wh_{sfx}"], np.float32)
        bh = np.asarray(inputs[f"bh_{sfx}"], np.float32)
        wrec = np.stack([_lay_w(Wh[0]), _lay_w(Wx[1]), _lay_w(Wh[1]),
                         _lay_w(Wx[2]), _lay_w(Wh[2])])  # [5, P, KC, H]
        bias0 = (bx[0] + bh[0]).reshape(KC, P).T.astype(np.float32)  # [P, KC]
        bias12 = np.stack([(bx[1] + bh[1]).reshape(KC, P).T,
                           (bx[2] + bh[2]).reshape(KC, P).T], 1).astype(np.float32)
        dirs[d] = dict(
            x=xx,
            wx0=_lay_w(Wx[0]),
            wrec=np.ascontiguousarray(wrec),
            bias0=np.ascontiguousarray(bias0),
            bias12=np.ascontiguousarray(bias12),
            wfc=_lay_w(wfc_half),
            fcb=fcb.reshape(NCLS, 1).astype(np.float32),
        )

    n_shard = N_CORES // 2
    in_maps = []
    for core in range(N_CORES):
        d = 0 if core < n_shard else 1
        s = core % n_shard
        dd = dirs[d]
        xc = dd["x"][s * B:(s + 1) * B]  # [B, T, IN]
        xa = np.zeros((3 * P, T * B), np.float32)
        xa[:IN_DIM] = xc.transpose(2, 1, 0).reshape(IN_DIM, T * B)
        xa[IN_DIM] = 1.0
        in_maps.append({
            "xt": np.ascontiguousarray(xa.reshape(3, P, T * B)).astype(NP16),
            "wemb": wemb_lay,
            "wx0": dd["wx0"],
            "wrec": dd["wrec"],
            "bias0": dd["bias0"],
            "bias12": dd["bias12"],
            "wfc": dd["wfc"],
            "fcb": dd["fcb"],
        })
    return in_maps


def combine_outputs(results, pad_start_index, T=256, B=32):
    """results: list of 8 dicts with 'out' [NCLS, T*B]. Returns [128*T, NCLS]."""
    n_shard = N_CORES // 2
    Bfull = n_shard * B
    L = np.zeros((2, Bfull, T, NCLS), np.float32)
    for core in range(N_CORES):
        d = 0 if core < n_shard else 1
        s = core % n_shard
        o = results[core]["out"].reshape(NCLS, T, B)  # col = t*B + b
        L[d, s * B:(s + 1) * B] = o.transpose(2, 1, 0)
    p = np.asarray(pad_start_index).astype(np.int64)[:, None]
    j = np.arange(T)[None, :]
    idx = np.where(j < p, p - j - 1, j)  # [Bfull, T]
    L2g = np.take_along_axis(L[1], idx[:, :, None], axis=1)
    logits = L[0] + L2g
    return logits.reshape(Bfull * T, NCLS)


_NC_CACHE = {}


def kernel(**inputs) -> np.ndarray:
    T = int(inputs["max_length"])
    assert T == 256, f"kernel compiled for T=256, got {T}"
    B = 32
    ln_g = np.asarray(inputs["ln_g"], np.float32)
    ln_b = np.asarray(inputs["ln_b"], np.float32)
    assert np.all(ln_g == 1.0) and np.all(ln_b == 0.0), \
        "general ln_g/ln_b path not wired"

    key = (T, B)
    if key not in _NC_CACHE:
        _NC_CACHE[key] = build_nc(T=T, B=B)
    nc = _NC_CACHE[key]

    in_maps = make_in_maps(inputs, T=T, B=B)
    res = run_bass_kernel_spmd(nc, in_maps, list(range(N_CORES)))
    return combine_outputs(res.results, inputs["pad_start_index"], T=T, B=B)


if __name__ == "__main__":
    import reference
    inp = reference.setup_inputs()
    out = kernel(**{k: np.asarray(v) for k, v in inp.items()})
    ref = np.asarray(reference.reference(**inp))
    err = np.abs(out - ref).max() / np.abs(ref).max()
    print(f"Relative error: {err:.3e}")


# revision 16
# speedup vs baseline: 1.2540x; 1.1686x over previous
"""Trainium2 Bass kernel for nn_BidirectionalRNN (3-layer LN-tanh RNN, bidir).

Sharding: 8 cores = 4 batch-shards x 2 directions (l2r on cores 0-3, r2l on
cores 4-7), B_loc=32 batches per core. All cores run the same SPMD program;
per-core inputs differ (direction weights + batch slice).

On-core layout: everything "transposed" — H on partitions as 4 chunks of 128,
batch along free dim. State h_l is one SBUF tile [128, 4, 32] (chunk-major).
Per step/layer:
  psum_pre[128,4,32] = Wh_l.T @ h_l(t-1) + Wx_l.T @ h_{l-1}(t)   (bf16 matmuls)
  s  = psum_pre + bias  (layer0: + xproj[t], bias prefolded)  -> bf16 st tile
  s2 = s*s                                                     -> st cols B:2B
  stats[1,2B] = (ones/512).T @ st  (PE, per k-chunk accumulate)  = [mean|meansq]
  m2 = Square(mean) (ACT); var = meansq - m2 (DVE)
  rstd = pow(var + eps, -0.5) (DVE tensor_scalar dual-op); c = mean*rstd
  [A|C][128,2B] = ones16.T @ [rstd|c]  (PE broadcast, fp16)
  y = s*A - C ; h_l = Tanh(y) (ACT, out bf16)
Embedding + xproj0 for layer0 are precomputed into SBUF (xp tile, bf16) by a
slab-wise pre-phase: xemb.T = Wemb_aug.T @ xT_aug (bias via appended ones row),
xproj = Wx0.T @ xemb.T + (bx0+bh0).
Final FC is accumulated per step on the PE: L[45, t*B:(t+1)*B] = Wfc_half.T @ h2
(+ b_fc on the l2r cores only, via per-core input).
Host combines: logits[b,t] = L_l2r[b,t] + L_r2l[b, idx[b,t]] (time gather
commutes with the channel-wise FC).
"""

import numpy as np
import ml_dtypes

import concourse.bass as bass
import concourse.bacc as bacc
import concourse.tile as tile
from concourse import mybir
from concourse.bass_utils import run_bass_kernel_spmd

BF16 = ml_dtypes.bfloat16
import os
USE_FP16 = os.environ.get("KERNEL_DT16", "f16") == "f16"
N_NEWTON = int(os.environ.get("KERNEL_NEWTON", "1"))
NP16 = np.float16 if USE_FP16 else BF16

H = 512
IN_DIM = 300
NCLS = 45
LN_EPS = 1e-5
P = 128
KC = H // P  # 4 chunks
N_CORES = 8

f32 = mybir.dt.float32
f16 = mybir.dt.float16
bf16 = mybir.dt.float16 if USE_FP16 else mybir.dt.bfloat16


def _stride0_view(ap, reps, width):
    """[P, width] AP -> [P, reps, width] AP re-reading the same cols."""
    return bass.AP(tensor=ap.tensor, offset=ap.offset,
                   ap=[ap.ap[0], [0, reps], [1, width]])


def build_nc(T=256, B=32, apply_gb=False):
    """Build the SPMD program. B = batches per core."""
    COLS = T * B
    S = min(1024, COLS)          # pre-phase slab width (cols)
    MMN = min(512, S)            # matmul moving width
    n_slabs = COLS // S

    nc = bacc.Bacc(None, target_bir_lowering=False)

    # ---- DRAM parameters (per-core values supplied via in_maps) ----
    xt_d = nc.dram_tensor("xt", [3, P, COLS], bf16, kind="ExternalInput")
    wemb_d = nc.dram_tensor("wemb", [P, 3, H], bf16, kind="ExternalInput")
    wx0_d = nc.dram_tensor("wx0", [P, KC, H], bf16, kind="ExternalInput")
    # recurrence weights: Wh0, Wx1, Wh1, Wx2, Wh2
    wrec_d = nc.dram_tensor("wrec", [5, P, KC, H], bf16, kind="ExternalInput")
    bias0_d = nc.dram_tensor("bias0", [P, KC], f32, kind="ExternalInput")
    bias12_d = nc.dram_tensor("bias12", [P, 2, KC], f32, kind="ExternalInput")
    wfc_d = nc.dram_tensor("wfc", [P, KC, NCLS], bf16, kind="ExternalInput")
    fcb_d = nc.dram_tensor("fcb", [NCLS, 1], f32, kind="ExternalInput")
    if apply_gb:
        gb_d = nc.dram_tensor("gb", [P, 3, 2, KC], f32, kind="ExternalInput")
    out_d = nc.dram_tensor("out", [NCLS, COLS], f32, kind="ExternalOutput")

    with tile.TileContext(nc) as tc:
        import contextlib
        with contextlib.ExitStack() as ctx:
            const = ctx.enter_context(tc.tile_pool(name="const", bufs=1))
            big = ctx.enter_context(tc.tile_pool(name="big", bufs=1))
            xtp = ctx.enter_context(tc.tile_pool(name="xtp", bufs=2))
            xep = ctx.enter_context(tc.tile_pool(name="xep", bufs=2))
            stp = ctx.enter_context(tc.tile_pool(name="stp", bufs=3))
            hp = ctx.enter_context(tc.tile_pool(name="hp", bufs=3))
            yp = ctx.enter_context(tc.tile_pool(name="yp", bufs=3))
            tiny = ctx.enter_context(tc.tile_pool(name="tiny", bufs=4))
            ps_pre = ctx.enter_context(tc.tile_pool(name="ps_pre", bufs=3, space="PSUM"))
            ps_st = ctx.enter_context(tc.tile_pool(name="ps_st", bufs=2, space="PSUM"))
            ps_bc = ctx.enter_context(tc.tile_pool(name="ps_bc", bufs=2, space="PSUM"))
            ps_l = ctx.enter_context(tc.tile_pool(name="ps_l", bufs=1, space="PSUM"))
            ps_bp = ps_pre

            # ---- constants / weights into SBUF ----
            wemb_sb = const.tile([P, 3, H], bf16)
            nc.sync.dma_start(out=wemb_sb, in_=wemb_d.ap())
            wx0_sb = const.tile([P, KC, H], bf16)
            nc.sync.dma_start(out=wx0_sb, in_=wx0_d.ap())
            wrec_sb = const.tile([P, 5, KC, H], bf16)
            nc.sync.dma_start(out=wrec_sb, in_=wrec_d.ap().rearrange("n p k m -> p n k m"))
            bias0_sb = const.tile([P, KC], f32)
            nc.sync.dma_start(out=bias0_sb, in_=bias0_d.ap())
            wfc_sb = const.tile([P, KC, NCLS], bf16)
            nc.sync.dma_start(out=wfc_sb, in_=wfc_d.ap())
            fcb_sb = const.tile([NCLS, 1], f32)
            nc.sync.dma_start(out=fcb_sb, in_=fcb_d.ap())
            if apply_gb:
                gb_sb = const.tile([P, 3, 2, KC], f32)
                nc.sync.dma_start(out=gb_sb, in_=gb_d.ap())

            ones16 = const.tile([1, P], f16)
            nc.vector.memset(ones16, 1.0)
            sc_ones = const.tile([P, 1], f16)
            nc.vector.memset(sc_ones, 1.0 / H)
            qk32 = const.tile([1, B], mybir.dt.int32)
            nc.vector.memset(qk32, 0x5F3759DF)
            bias12_sb = const.tile([P, 2, KC], f32)
            nc.sync.dma_start(out=bias12_sb, in_=bias12_d.ap())

            xp_sb = big.tile([P, T, KC, B], bf16)     # xproj0 (+bias0), all steps
            L_sb = big.tile([NCLS, COLS], f32)        # FC accumulator

            # ---- pre-phase: embedding + xproj0, slab by slab ----
            for sl in range(n_slabs):
                c0 = sl * S
                xt_tiles = []
                for k in range(3):
                    xt_t = xtp.tile([P, S], bf16, tag=f"xt{k}")
                    nc.sync.dma_start(out=xt_t, in_=xt_d.ap()[k, :, c0:c0 + S])
                    xt_tiles.append(xt_t)
                xe_tiles = []
                for m in range(KC):
                    xe_t = xep.tile([P, S], bf16, tag=f"xe{m}")
                    xe_tiles.append(xe_t)
                for m in range(KC):
                    for ns in range(S // MMN):
                        pse = ps_bp.tile([P, MMN], f32, tag="pre")
                        for k in range(3):
                            nc.tensor.matmul(pse, wemb_sb[:, k, bass.ts(m, P)],
                                             xt_tiles[k][:, bass.ts(ns, MMN)],
                                             start=(k == 0), stop=(k == 2))
                        nc.scalar.copy(xe_tiles[m][:, bass.ts(ns, MMN)], pse)
                for m in range(KC):
                    for ns in range(S // MMN):
                        psx = ps_bp.tile([P, MMN], f32, tag="pre")
                        for k in range(KC):
                            nc.tensor.matmul(psx, wx0_sb[:, k, bass.ts(m, P)],
                                             xe_tiles[k][:, bass.ts(ns, MMN)],
                                             start=(k == 0), stop=(k == KC - 1))
                        n0 = c0 + ns * MMN
                        t0 = n0 // B
                        nt = MMN // B
                        nc.vector.tensor_scalar(
                            xp_sb[:, t0:t0 + nt, m, :], psx,
                            bias0_sb[:, m:m + 1], None, mybir.AluOpType.add)

            # ---- recurrence (wavefront emission) ----
            h = []
            for l in range(3):
                h0 = hp.tile([P, KC, B], bf16, tag=f"h{l}")
                nc.vector.memset(h0, 0.0)
                h.append(h0)

            wh_idx = [0, 2, 4]   # Wh0, Wh1, Wh2 in wrec
            wx_idx = [None, 1, 3]
            i32 = mybir.dt.int32
            Sq = mybir.ActivationFunctionType.Square

            def emit_unit(t, l):
                ps = ps_pre.tile([P, KC, B], f32, tag="pre")
                for m in range(KC):
                    n_mm = KC * (2 if l > 0 else 1)
                    i = 0
                    for k in range(KC):
                        nc.tensor.matmul(ps[:, m, :],
                                         wrec_sb[:, wh_idx[l], k, bass.ts(m, P)],
                                         h[l][:, k, :],
                                         start=(i == 0), stop=(i == n_mm - 1))
                        i += 1
                    if l > 0:
                        for k in range(KC):
                            nc.tensor.matmul(ps[:, m, :],
                                             wrec_sb[:, wx_idx[l], k, bass.ts(m, P)],
                                             h[l - 1][:, k, :],
                                             start=False, stop=(i == n_mm - 1))
                            i += 1

                st = stp.tile([P, KC, 2 * B], bf16, tag="st")
                if l == 0:
                    nc.vector.tensor_tensor(st[:, :, :B], ps, xp_sb[:, t, :, :],
                                            mybir.AluOpType.add)
                else:
                    nc.vector.tensor_tensor(
                        st[:, :, :B], ps,
                        _stride0_view_mid(bias12_sb[:, l - 1, :], B),
                        mybir.AluOpType.add)
                nc.scalar.activation(st[:, :, B:], st[:, :, :B], Sq)

                # stats as rows on partition 0 (ones stationary, psum k-fold)
                pst = ps_st.tile([1, 2, B], f32, tag="pst")
                for k in range(KC):
                    nc.tensor.matmul(pst[0:1, 0, :], sc_ones, st[:, k, :B],
                                     start=(k == 0), stop=(k == KC - 1))
                for k in range(KC):
                    nc.tensor.matmul(pst[0:1, 1, :], sc_ones, st[:, k, B:],
                                     start=(k == 0), stop=(k == KC - 1))

                m2 = tiny.tile([1, B], f32, tag="m2")
                nc.scalar.activation(m2, pst[0:1, 0, :], Sq)
                ve = tiny.tile([1, B], f32, tag="ve")
                nc.vector.tensor_tensor(ve, pst[0:1, 1, :], m2,
                                        mybir.AluOpType.subtract)
                nc.vector.tensor_scalar(ve, ve, LN_EPS, None,
                                        mybir.AluOpType.add)
                ui = tiny.tile([1, B], i32, tag="ui")
                nc.vector.tensor_scalar(ui, ve.bitcast(i32), 1, None,
                                        mybir.AluOpType.arith_shift_right)
                y0i = tiny.tile([1, B], i32, tag="y0i")
                nc.vector.tensor_tensor(y0i, qk32, ui, mybir.AluOpType.subtract)
                cur = y0i.bitcast(f32)
                rc16 = tiny.tile([1, 2 * B], f16, tag="rc16")
                for it in range(N_NEWTON):
                    y2 = tiny.tile([1, B], f32, tag=f"nw_y2_{it}")
                    nc.vector.tensor_tensor(y2, cur, cur, mybir.AluOpType.mult)
                    xy2 = tiny.tile([1, B], f32, tag=f"nw_xy2_{it}")
                    nc.vector.tensor_tensor(xy2, ve, y2, mybir.AluOpType.mult)
                    e = tiny.tile([1, B], f32, tag=f"nw_e_{it}")
                    nc.vector.tensor_scalar(e, xy2, -0.5, 1.5,
                                            mybir.AluOpType.mult,
                                            mybir.AluOpType.add)
                    if it < N_NEWTON - 1:
                        yn = tiny.tile([1, B], f32, tag=f"nw_yn_{it}")
                        nc.vector.tensor_tensor(yn, cur, e, mybir.AluOpType.mult)
                        cur = yn
                    else:
                        nc.vector.tensor_tensor(rc16[:, 0:B], cur, e,
                                                mybir.AluOpType.mult)
                cm = tiny.tile([1, B], f32, tag="cm")
                nc.vector.tensor_tensor(cm, pst[0:1, 0, :], cur,
                                        mybir.AluOpType.mult)
                nc.vector.tensor_tensor(rc16[:, B:2 * B], cm, e,
                                        mybir.AluOpType.mult)

                # broadcast A/C across partitions, KC-expanded via stride-0 moving
                psac = ps_bc.tile([P, 2, KC, B], f32, tag="bc")
                av = rc16[:, 0:B]
                av = bass.AP(tensor=av.tensor, offset=av.offset,
                             ap=[av.ap[0], [0, KC], [1, B]])
                cv = rc16[:, B:2 * B]
                cv = bass.AP(tensor=cv.tensor, offset=cv.offset,
                             ap=[cv.ap[0], [0, KC], [1, B]])
                nc.tensor.matmul(psac[:, 0, :, :], ones16, av,
                                 start=True, stop=True)
                nc.tensor.matmul(psac[:, 1, :, :], ones16, cv,
                                 start=True, stop=True)

                y = yp.tile([P, KC, B], f16, tag="y")
                nc.vector.tensor_tensor(y, st[:, :, :B], psac[:, 0, :, :],
                                        mybir.AluOpType.mult)
                nc.vector.tensor_tensor(y, y, psac[:, 1, :, :],
                                        mybir.AluOpType.subtract)
                if apply_gb:
                    nc.vector.tensor_tensor(
                        y, y, _stride0_view_mid(gb_sb[:, l, 0, :], B),
                        mybir.AluOpType.mult)
                    nc.vector.tensor_tensor(
                        y, y, _stride0_view_mid(gb_sb[:, l, 1, :], B),
                        mybir.AluOpType.add)
                hn = hp.tile([P, KC, B], bf16, tag=f"h{l}")
                nc.scalar.activation(hn, y, mybir.ActivationFunctionType.Tanh)
                h[l] = hn
                if l == 2:
                    psl = ps_l.tile([NCLS, B], f32, tag="L")
                    for k in range(KC):
                        nc.tensor.matmul(psl, wfc_sb[:, k, :], h[2][:, k, :],
                                         start=(k == 0), stop=(k == KC - 1))
                    nc.vector.tensor_scalar(L_sb[:, t * B:(t + 1) * B], psl,
                                            fcb_sb, None, mybir.AluOpType.add)

            for tau in range(T + 2):
                for l in (2, 1, 0):
                    t = tau - l
                    if 0 <= t < T:
                        emit_unit(t, l)

            nc.sync.dma_start(out=out_d.ap(), in_=L_sb)

    nc.compile()
    return nc


def _stride0_view_mid(ap, width):
    """[P, KC] AP -> [P, KC, width] AP, broadcasting each col along width."""
    return bass.AP(tensor=ap.tensor, offset=ap.offset,
                   ap=[ap.ap[0], ap.ap[1], [0, width]])


# ---------------- host-side prep ----------------

def _lay_w(w):
    """[H, M] fp32 -> [P, KC, M] bf16 chunk layout."""
    Hh, M = w.shape
    kc = Hh // P
    return np.ascontiguousarray(
        w.reshape(kc, P, M).transpose(1, 0, 2)).astype(NP16)


def make_in_maps(inputs, T=256, B=32):
    """Build the 8 per-core input dicts from the full problem inputs."""
    x = np.asarray(inputs["x"], np.float32)[:, :T]
    rx = np.asarray(inputs["reverse_x"], np.float32)[:, :T]
    W_emb = np.asarray(inputs["W_emb"], np.float32)
    b_emb = np.asarray(inputs["b_emb"], np.float32)
    W_fc = np.asarray(inputs["W_fc"], np.float32)
    b_fc = np.asarray(inputs["b_fc"], np.float32)

    wemb_aug = np.zeros((3 * P, H), np.float32)
    wemb_aug[:IN_DIM] = W_emb
    wemb_aug[IN_DIM] = b_emb
    wemb_lay = _lay_w(wemb_aug)  # [P, 3, H]

    dirs = {}
    for d, (xx, sfx, wfc_half, fcb) in enumerate([
            (x, "l2r", W_fc[:H], b_fc),
            (rx, "r2l", W_fc[H:], np.zeros_like(b_fc))]):
        Wx = np.asarray(inputs[f"Wx_{sfx}"], np.float32)
        bx = np.asarray(inputs[f"bx_{sfx}"], np.float32)
        Wh = np.asarray(inputs[f"Wh_{sfx}"], np.float32)
        bh = np.asarray(inputs[f"bh_{sfx}"], np.float32)
        wrec = np.stack([_lay_w(Wh[0]), _lay_w(Wx[1]), _lay_w(Wh[1]),
                         _lay_w(Wx[2]), _lay_w(Wh[2])])  # [5, P, KC, H]
        bias0 = (bx[0] + bh[0]).reshape(KC, P).T.astype(np.float32)  # [P, KC]
        bias12 = np.stack([(bx[1] + bh[1]).reshape(KC, P).T,
                           (bx[2] + bh[2]).reshape(KC, P).T], 1).astype(np.float32)
        dirs[d] = dict(
            x=xx,
            wx0=_lay_w(Wx[0]),
            wrec=np.ascontiguousarray(wrec),
            bias0=np.ascontiguousarray(bias0),
            bias12=np.ascontiguousarray(bias12),
            wfc=_lay_w(wfc_half),
            fcb=fcb.reshape(NCLS, 1).astype(np.float32),
        )

    n_shard = N_CORES // 2
    in_maps = []
    for core in range(N_CORES):
        d = 0 if core < n_shard else 1
        s = core % n_shard
        dd = dirs[d]
        xc = dd["x"][s * B:(s + 1) * B]  # [B, T, IN]
        xa = np.zeros((3 * P, T * B), np.float32)
        xa[:IN_DIM] = xc.transpose(2, 1, 0).reshape(IN_DIM, T * B)
        xa[IN_DIM] = 1.0
        in_maps.append({
            "xt": np.ascontiguousarray(xa.reshape(3, P, T * B)).astype(NP16),
            "wemb": wemb_lay,
            "wx0": dd["wx0"],
            "wrec": dd["wrec"],
            "bias0": dd["bias0"],
            "bias12": dd["bias12"],
            "wfc": dd["wfc"],
            "fcb": dd["fcb"],
        })
    return in_maps


def combine_outputs(results, pad_start_index, T=256, B=32):
    """results: list of 8 dicts with 'out' [NCLS, T*B]. Returns [128*T, NCLS]."""
    n_shard = N_CORES // 2
    Bfull = n_shard * B
    L = np.zeros((2, Bfull, T, NCLS), np.float32)
    for core in range(N_CORES):
        d = 0 if core < n_shard else 1
        s = core % n_shard
        o = results[core]["out"].reshape(NCLS, T, B)  # col = t*B + b
        L[d, s * B:(s + 1) * B] = o.transpose(2, 1, 0)
    p = np.asarray(pad_start_index).astype(np.int64)[:, None]
    j = np.arange(T)[None, :]
    idx = np.where(j < p, p - j - 1, j)  # [Bfull, T]
    L2g = np.take_along_axis(L[1], idx[:, :, None], axis=1)
    logits = L[0] + L2g
    return logits.reshape(Bfull * T, NCLS)


_NC_CACHE = {}


def kernel(**inputs) -> np.ndarray:
    T = int(inputs["max_length"])
    assert T == 256, f"kernel compiled for T=256, got {T}"
    B = 32
    ln_g = np.asarray(inputs["ln_g"], np.float32)
    ln_b = np.asarray(inputs["ln_b"], np.float32)
    apply_gb = not (np.all(ln_g == 1.0) and np.all(ln_b == 0.0))
    assert not apply_gb, "general ln_g/ln_b path not wired yet"

    key = (T, B, apply_gb)
    if key not in _NC_CACHE:
        _NC_CACHE[key] = build_nc(T=T, B=B, apply_gb=apply_gb)
    nc = _NC_CACHE[key]

    in_maps = make_in_maps(inputs, T=T, B=B)
    res = run_bass_kernel_spmd(nc, in_maps, list(range(N_CORES)))
    return combine_outputs(res.results, inputs["pad_start_index"], T=T, B=B)


if __name__ == "__main__":
    import reference
    inp = reference.setup_inputs()
    out = kernel(**{k: np.asarray(v) for k, v in inp.items()})
    ref = np.asarray(reference.reference(**inp))
    err = np.abs(out - ref).max() / np.abs(ref).max()
    print(f"Relative error: {err:.3e}")



# revision 17
# speedup vs baseline: 1.2815x; 1.0219x over previous
"""Trainium2 Bass kernel for nn_BidirectionalRNN (3-layer LN-tanh RNN, bidir).

Sharding: 8 cores = 4 batch-shards x 2 directions (l2r on cores 0-3, r2l on
cores 4-7), B_loc=32 batches per core. All cores run the same SPMD program;
per-core inputs differ (direction weights + batch slice).

On-core layout: everything "transposed" — H on partitions as 4 chunks of 128,
batch along free dim. State h_l is one SBUF tile [128, 4, 32] (chunk-major).
Per step/layer:
  psum_pre[128,4,32] = Wh_l.T @ h_l(t-1) + Wx_l.T @ h_{l-1}(t)   (bf16 matmuls)
  s  = psum_pre + bias  (layer0: + xproj[t], bias prefolded)  -> bf16 st tile
  s2 = s*s                                                     -> st cols B:2B
  stats[1,2B] = (ones/512).T @ st  (PE, per k-chunk accumulate)  = [mean|meansq]
  m2 = Square(mean) (ACT); var = meansq - m2 (DVE)
  rstd = pow(var + eps, -0.5) (DVE tensor_scalar dual-op); c = mean*rstd
  [A|C][128,2B] = ones16.T @ [rstd|c]  (PE broadcast, fp16)
  y = s*A - C ; h_l = Tanh(y) (ACT, out bf16)
Embedding + xproj0 for layer0 are precomputed into SBUF (xp tile, bf16) by a
slab-wise pre-phase: xemb.T = Wemb_aug.T @ xT_aug (bias via appended ones row),
xproj = Wx0.T @ xemb.T + (bx0+bh0).
Final FC is accumulated per step on the PE: L[45, t*B:(t+1)*B] = Wfc_half.T @ h2
(+ b_fc on the l2r cores only, via per-core input).
Host combines: logits[b,t] = L_l2r[b,t] + L_r2l[b, idx[b,t]] (time gather
commutes with the channel-wise FC).
"""

import numpy as np
import ml_dtypes

import concourse.bass as bass
import concourse.bacc as bacc
import concourse.tile as tile
from concourse import mybir
from concourse.bass_utils import run_bass_kernel_spmd

BF16 = ml_dtypes.bfloat16
import os
USE_FP16 = os.environ.get("KERNEL_DT16", "f16") == "f16"
N_NEWTON = int(os.environ.get("KERNEL_NEWTON", "1"))
NP16 = np.float16 if USE_FP16 else BF16

H = 512
IN_DIM = 300
NCLS = 45
LN_EPS = 1e-5
P = 128
KC = H // P  # 4 chunks
N_CORES = 8

f32 = mybir.dt.float32
f16 = mybir.dt.float16
bf16 = mybir.dt.float16 if USE_FP16 else mybir.dt.bfloat16


def _stride0_view(ap, reps, width):
    """[P, width] AP -> [P, reps, width] AP re-reading the same cols."""
    return bass.AP(tensor=ap.tensor, offset=ap.offset,
                   ap=[ap.ap[0], [0, reps], [1, width]])


def build_nc(T=256, B=32, apply_gb=False):
    """Build the SPMD program. B = batches per core."""
    COLS = T * B
    S = min(1024, COLS)          # pre-phase slab width (cols)
    MMN = min(512, S)            # matmul moving width
    n_slabs = COLS // S

    nc = bacc.Bacc(None, target_bir_lowering=False)

    # ---- DRAM parameters (per-core values supplied via in_maps) ----
    xt_d = nc.dram_tensor("xt", [3, P, COLS], bf16, kind="ExternalInput")
    wemb_d = nc.dram_tensor("wemb", [P, 3, H], bf16, kind="ExternalInput")
    wx0_d = nc.dram_tensor("wx0", [P, KC, H], bf16, kind="ExternalInput")
    # recurrence weights: Wh0, Wx1, Wh1, Wx2, Wh2
    wrec_d = nc.dram_tensor("wrec", [5, P, KC, H], bf16, kind="ExternalInput")
    bias0_d = nc.dram_tensor("bias0", [P, KC], f32, kind="ExternalInput")
    bias12_d = nc.dram_tensor("bias12", [P, 2, KC], f32, kind="ExternalInput")
    wfc_d = nc.dram_tensor("wfc", [P, KC, NCLS], bf16, kind="ExternalInput")
    fcb_d = nc.dram_tensor("fcb", [NCLS, 1], f32, kind="ExternalInput")
    if apply_gb:
        gb_d = nc.dram_tensor("gb", [P, 3, 2, KC], f32, kind="ExternalInput")
    out_d = nc.dram_tensor("out", [NCLS, COLS], f32, kind="ExternalOutput")

    with tile.TileContext(nc) as tc:
        import contextlib
        with contextlib.ExitStack() as ctx:
            const = ctx.enter_context(tc.tile_pool(name="const", bufs=1))
            big = ctx.enter_context(tc.tile_pool(name="big", bufs=1))
            xtp = ctx.enter_context(tc.tile_pool(name="xtp", bufs=2))
            xep = ctx.enter_context(tc.tile_pool(name="xep", bufs=2))
            stp = ctx.enter_context(tc.tile_pool(name="stp", bufs=3))
            hp = ctx.enter_context(tc.tile_pool(name="hp", bufs=3))
            yp = ctx.enter_context(tc.tile_pool(name="yp", bufs=3))
            tiny = ctx.enter_context(tc.tile_pool(name="tiny", bufs=4))
            ps_pre = ctx.enter_context(tc.tile_pool(name="ps_pre", bufs=3, space="PSUM"))
            ps_st = ctx.enter_context(tc.tile_pool(name="ps_st", bufs=2, space="PSUM"))
            ps_bc = ctx.enter_context(tc.tile_pool(name="ps_bc", bufs=2, space="PSUM"))
            ps_l = ctx.enter_context(tc.tile_pool(name="ps_l", bufs=1, space="PSUM"))
            ps_bp = ps_pre

            # ---- constants / weights into SBUF ----
            wemb_sb = const.tile([P, 3, H], bf16)
            nc.sync.dma_start(out=wemb_sb, in_=wemb_d.ap())
            wx0_sb = const.tile([P, KC, H], bf16)
            nc.sync.dma_start(out=wx0_sb, in_=wx0_d.ap())
            wrec_sb = const.tile([P, 5, KC, H], bf16)
            nc.sync.dma_start(out=wrec_sb, in_=wrec_d.ap().rearrange("n p k m -> p n k m"))
            bias0_sb = const.tile([P, KC], f32)
            nc.sync.dma_start(out=bias0_sb, in_=bias0_d.ap())
            wfc_sb = const.tile([P, KC, NCLS], bf16)
            nc.sync.dma_start(out=wfc_sb, in_=wfc_d.ap())
            fcb_sb = const.tile([NCLS, 1], f32)
            nc.sync.dma_start(out=fcb_sb, in_=fcb_d.ap())
            if apply_gb:
                gb_sb = const.tile([P, 3, 2, KC], f32)
                nc.sync.dma_start(out=gb_sb, in_=gb_d.ap())

            ones16 = const.tile([1, P], f16)
            nc.vector.memset(ones16, 1.0)
            sc_ones = const.tile([P, 1], f16)
            nc.vector.memset(sc_ones, 1.0 / H)
            qk32 = const.tile([1, B], mybir.dt.int32)
            nc.vector.memset(qk32, 0x5F3759DF)
            bias12_sb = const.tile([P, 2, KC], f32)
            nc.sync.dma_start(out=bias12_sb, in_=bias12_d.ap())

            xp_sb = big.tile([P, T, KC, B], bf16)     # xproj0 (+bias0), all steps
            L_sb = big.tile([NCLS, COLS], f32)        # FC accumulator

            # ---- pre-phase: embedding + xproj0, slab by slab ----
            for sl in range(n_slabs):
                c0 = sl * S
                xt_tiles = []
                for k in range(3):
                    xt_t = xtp.tile([P, S], bf16, tag=f"xt{k}")
                    nc.sync.dma_start(out=xt_t, in_=xt_d.ap()[k, :, c0:c0 + S])
                    xt_tiles.append(xt_t)
                xe_tiles = []
                for m in range(KC):
                    xe_t = xep.tile([P, S], bf16, tag=f"xe{m}")
                    xe_tiles.append(xe_t)
                for m in range(KC):
                    for ns in range(S // MMN):
                        pse = ps_bp.tile([P, MMN], f32, tag="pre")
                        for k in range(3):
                            nc.tensor.matmul(pse, wemb_sb[:, k, bass.ts(m, P)],
                                             xt_tiles[k][:, bass.ts(ns, MMN)],
                                             start=(k == 0), stop=(k == 2))
                        nc.scalar.copy(xe_tiles[m][:, bass.ts(ns, MMN)], pse)
                for m in range(KC):
                    for ns in range(S // MMN):
                        psx = ps_bp.tile([P, MMN], f32, tag="pre")
                        for k in range(KC):
                            nc.tensor.matmul(psx, wx0_sb[:, k, bass.ts(m, P)],
                                             xe_tiles[k][:, bass.ts(ns, MMN)],
                                             start=(k == 0), stop=(k == KC - 1))
                        n0 = c0 + ns * MMN
                        t0 = n0 // B
                        nt = MMN // B
                        nc.vector.tensor_scalar(
                            xp_sb[:, t0:t0 + nt, m, :], psx,
                            bias0_sb[:, m:m + 1], None, mybir.AluOpType.add)

            # ---- recurrence (wavefront emission) ----
            h = []
            for l in range(3):
                h0 = hp.tile([P, KC, B], bf16, tag=f"h{l}")
                nc.vector.memset(h0, 0.0)
                h.append(h0)

            wh_idx = [0, 2, 4]   # Wh0, Wh1, Wh2 in wrec
            wx_idx = [None, 1, 3]
            i32 = mybir.dt.int32
            Sq = mybir.ActivationFunctionType.Square

            def emit_unit(t, l):
                ps = ps_pre.tile([P, KC, B], f32, tag="pre")
                for m in range(KC):
                    n_mm = KC * (2 if l > 0 else 1)
                    i = 0
                    for k in range(KC):
                        nc.tensor.matmul(ps[:, m, :],
                                         wrec_sb[:, wh_idx[l], k, bass.ts(m, P)],
                                         h[l][:, k, :],
                                         start=(i == 0), stop=(i == n_mm - 1))
                        i += 1
                    if l > 0:
                        for k in range(KC):
                            nc.tensor.matmul(ps[:, m, :],
                                             wrec_sb[:, wx_idx[l], k, bass.ts(m, P)],
                                             h[l - 1][:, k, :],
                                             start=False, stop=(i == n_mm - 1))
                            i += 1

                st = stp.tile([P, KC, 2 * B], bf16, tag="st")
                if l == 0:
                    nc.vector.tensor_tensor(st[:, :, :B], ps, xp_sb[:, t, :, :],
                                            mybir.AluOpType.add)
                else:
                    nc.vector.tensor_tensor(
                        st[:, :, :B], ps,
                        _stride0_view_mid(bias12_sb[:, l - 1, :], B),
                        mybir.AluOpType.add)
                nc.scalar.activation(st[:, :, B:], st[:, :, :B], Sq)

                # stats as rows on partition 0 (ones stationary, psum k-fold)
                pst = ps_st.tile([1, 2, B], f32, tag="pst")
                for k in range(KC):
                    nc.tensor.matmul(pst[0:1, 0, :], sc_ones, st[:, k, :B],
                                     start=(k == 0), stop=(k == KC - 1))
                for k in range(KC):
                    nc.tensor.matmul(pst[0:1, 1, :], sc_ones, st[:, k, B:],
                                     start=(k == 0), stop=(k == KC - 1))

                m2 = tiny.tile([1, B], f32, tag="m2")
                nc.scalar.activation(m2, pst[0:1, 0, :], Sq)
                ve = tiny.tile([1, B], f32, tag="ve")
                nc.vector.tensor_tensor(ve, pst[0:1, 1, :], m2,
                                        mybir.AluOpType.subtract)
                nc.vector.tensor_scalar(ve, ve, LN_EPS, None,
                                        mybir.AluOpType.add)
                ui = tiny.tile([1, B], i32, tag="ui")
                nc.vector.tensor_scalar(ui, ve.bitcast(i32), 1, None,
                                        mybir.AluOpType.arith_shift_right)
                y0i = tiny.tile([1, B], i32, tag="y0i")
                nc.vector.tensor_tensor(y0i, qk32, ui, mybir.AluOpType.subtract)
                cur = y0i.bitcast(f32)
                rc16 = tiny.tile([1, 2 * B], f16, tag="rc16")
                for it in range(N_NEWTON):
                    y2 = tiny.tile([1, B], f32, tag=f"nw_y2_{it}")
                    nc.vector.tensor_tensor(y2, cur, cur, mybir.AluOpType.mult)
                    xy2 = tiny.tile([1, B], f32, tag=f"nw_xy2_{it}")
                    nc.vector.tensor_tensor(xy2, ve, y2, mybir.AluOpType.mult)
                    e = tiny.tile([1, B], f32, tag=f"nw_e_{it}")
                    nc.vector.tensor_scalar(e, xy2, -0.5, 1.5,
                                            mybir.AluOpType.mult,
                                            mybir.AluOpType.add)
                    if it < N_NEWTON - 1:
                        yn = tiny.tile([1, B], f32, tag=f"nw_yn_{it}")
                        nc.vector.tensor_tensor(yn, cur, e, mybir.AluOpType.mult)
                        cur = yn
                    else:
                        nc.vector.tensor_tensor(rc16[:, 0:B], cur, e,
                                                mybir.AluOpType.mult)
                cm = tiny.tile([1, B], f32, tag="cm")
                nc.vector.tensor_tensor(cm, pst[0:1, 0, :], cur,
                                        mybir.AluOpType.mult)
                nc.vector.tensor_tensor(rc16[:, B:2 * B], cm, e,
                                        mybir.AluOpType.mult)

                # broadcast A/C across partitions, KC-expanded via stride-0 moving
                psac = ps_bc.tile([P, 2, KC, B], f32, tag="bc")
                av = rc16[:, 0:B]
                av = bass.AP(tensor=av.tensor, offset=av.offset,
                             ap=[av.ap[0], [0, KC], [1, B]])
                cv = rc16[:, B:2 * B]
                cv = bass.AP(tensor=cv.tensor, offset=cv.offset,
                             ap=[cv.ap[0], [0, KC], [1, B]])
                nc.tensor.matmul(psac[:, 0, :, :], ones16, av,
                                 start=True, stop=True)
                nc.tensor.matmul(psac[:, 1, :, :], ones16, cv,
                                 start=True, stop=True)

                ac16 = tiny.tile([P, 2, KC, B], f16, tag="ac16")
                nc.scalar.copy(ac16, psac)
                y = yp.tile([P, KC, B], f16, tag="y")
                nc.vector.tensor_tensor(y, st[:, :, :B], ac16[:, 0, :, :],
                                        mybir.AluOpType.mult)
                nc.vector.tensor_tensor(y, y, ac16[:, 1, :, :],
                                        mybir.AluOpType.subtract)
                if apply_gb:
                    nc.vector.tensor_tensor(
                        y, y, _stride0_view_mid(gb_sb[:, l, 0, :], B),
                        mybir.AluOpType.mult)
                    nc.vector.tensor_tensor(
                        y, y, _stride0_view_mid(gb_sb[:, l, 1, :], B),
                        mybir.AluOpType.add)
                hn = hp.tile([P, KC, B], bf16, tag=f"h{l}")
                nc.scalar.activation(hn, y, mybir.ActivationFunctionType.Tanh)
                h[l] = hn
                if l == 2:
                    psl = ps_l.tile([NCLS, B], f32, tag="L")
                    for k in range(KC):
                        nc.tensor.matmul(psl, wfc_sb[:, k, :], h[2][:, k, :],
                                         start=(k == 0), stop=(k == KC - 1))
                    nc.vector.tensor_scalar(L_sb[:, t * B:(t + 1) * B], psl,
                                            fcb_sb, None, mybir.AluOpType.add)

            for tau in range(T + 2):
                for l in (2, 1, 0):
                    t = tau - l
                    if 0 <= t < T:
                        emit_unit(t, l)

            nc.sync.dma_start(out=out_d.ap(), in_=L_sb)

    nc.compile()
    return nc


def _stride0_view_mid(ap, width):
    """[P, KC] AP -> [P, KC, width] AP, broadcasting each col along width."""
    return bass.AP(tensor=ap.tensor, offset=ap.offset,
                   ap=[ap.ap[0], ap.ap[1], [0, width]])


# ---------------- host-side prep ----------------

def _lay_w(w):
    """[H, M] fp32 -> [P, KC, M] bf16 chunk layout."""
    Hh, M = w.shape
    kc = Hh // P
    return np.ascontiguousarray(
        w.reshape(kc, P, M).transpose(1, 0, 2)).astype(NP16)


def make_in_maps(inputs, T=256, B=32):
    """Build the 8 per-core input dicts from the full problem inputs."""
    x = np.asarray(inputs["x"], np.float32)[:, :T]
    rx = np.asarray(inputs["reverse_x"], np.float32)[:, :T]
    W_emb = np.asarray(inputs["W_emb"], np.float32)
    b_emb = np.asarray(inputs["b_emb"], np.float32)
    W_fc = np.asarray(inputs["W_fc"], np.float32)
    b_fc = np.asarray(inputs["b_fc"], np.float32)

    wemb_aug = np.zeros((3 * P, H), np.float32)
    wemb_aug[:IN_DIM] = W_emb
    wemb_aug[IN_DIM] = b_emb
    wemb_lay = _lay_w(wemb_aug)  # [P, 3, H]

    dirs = {}
    for d, (xx, sfx, wfc_half, fcb) in enumerate([
            (x, "l2r", W_fc[:H], b_fc),
            (rx, "r2l", W_fc[H:], np.zeros_like(b_fc))]):
        Wx = np.asarray(inputs[f"Wx_{sfx}"], np.float32)
        bx = np.asarray(inputs[f"bx_{sfx}"], np.float32)
        Wh = np.asarray(inputs[f"Wh_{sfx}"], np.float32)
        bh = np.asarray(inputs[f"bh_{sfx}"], np.float32)
        wrec = np.stack([_lay_w(Wh[0]), _lay_w(Wx[1]), _lay_w(Wh[1]),
                         _lay_w(Wx[2]), _lay_w(Wh[2])])  # [5, P, KC, H]
        bias0 = (bx[0] + bh[0]).reshape(KC, P).T.astype(np.float32)  # [P, KC]
        bias12 = np.stack([(bx[1] + bh[1]).reshape(KC, P).T,
                           (bx[2] + bh[2]).reshape(KC, P).T], 1).astype(np.float32)
        dirs[d] = dict(
            x=xx,
            wx0=_lay_w(Wx[0]),
            wrec=np.ascontiguousarray(wrec),
            bias0=np.ascontiguousarray(bias0),
            bias12=np.ascontiguousarray(bias12),
            wfc=_lay_w(wfc_half),
            fcb=fcb.reshape(NCLS, 1).astype(np.float32),
        )

    n_shard = N_CORES // 2
    in_maps = []
    for core in range(N_CORES):
        d = 0 if core < n_shard else 1
        s = core % n_shard
        dd = dirs[d]
        xc = dd["x"][s * B:(s + 1) * B]  # [B, T, IN]
        xa = np.zeros((3 * P, T * B), np.float32)
        xa[:IN_DIM] = xc.transpose(2, 1, 0).reshape(IN_DIM, T * B)
        xa[IN_DIM] = 1.0
        in_maps.append({
            "xt": np.ascontiguousarray(xa.reshape(3, P, T * B)).astype(NP16),
            "wemb": wemb_lay,
            "wx0": dd["wx0"],
            "wrec": dd["wrec"],
            "bias0": dd["bias0"],
            "bias12": dd["bias12"],
            "wfc": dd["wfc"],
            "fcb": dd["fcb"],
        })
    return in_maps


def combine_outputs(results, pad_start_index, T=256, B=32):
    """results: list of 8 dicts with 'out' [NCLS, T*B]. Returns [128*T, NCLS]."""
    n_shard = N_CORES // 2
    Bfull = n_shard * B
    L = np.zeros((2, Bfull, T, NCLS), np.float32)
    for core in range(N_CORES):
        d = 0 if core < n_shard else 1
        s = core % n_shard
        o = results[core]["out"].reshape(NCLS, T, B)  # col = t*B + b
        L[d, s * B:(s + 1) * B] = o.transpose(2, 1, 0)
    p = np.asarray(pad_start_index).astype(np.int64)[:, None]
    j = np.arange(T)[None, :]
    idx = np.where(j < p, p - j - 1, j)  # [Bfull, T]
    L2g = np.take_along_axis(L[1], idx[:, :, None], axis=1)
    logits = L[0] + L2g
    return logits.reshape(Bfull * T, NCLS)


_NC_CACHE = {}


def kernel(**inputs) -> np.ndarray:
    T = int(inputs["max_length"])
    assert T == 256, f"kernel compiled for T=256, got {T}"
    B = 32
    ln_g = np.asarray(inputs["ln_g"], np.float32)
    ln_b = np.asarray(inputs["ln_b"], np.float32)
    apply_gb = not (np.all(ln_g == 1.0) and np.all(ln_b == 0.0))
    assert not apply_gb, "general ln_g/ln_b path not wired yet"

    key = (T, B, apply_gb)
    if key not in _NC_CACHE:
        _NC_CACHE[key] = build_nc(T=T, B=B, apply_gb=apply_gb)
    nc = _NC_CACHE[key]

    in_maps = make_in_maps(inputs, T=T, B=B)
    res = run_bass_kernel_spmd(nc, in_maps, list(range(N_CORES)))
    return combine_outputs(res.results, inputs["pad_start_index"], T=T, B=B)


if __name__ == "__main__":
    import reference
    inp = reference.setup_inputs()
    out = kernel(**{k: np.asarray(v) for k, v in inp.items()})
    ref = np.asarray(reference.reference(**inp))
    err = np.abs(out - ref).max() / np.abs(ref).max()
    print(f"Relative error: {err:.3e}")

